# revision 17
# baseline (speedup 1.0000x reference)
"""Depthwise-masked 3x3 conv (eye-masked dense conv) on 8 TRN2 NeuronCores.

Problem: x (2,16,256,64,64) fp32, W (256,256,3,3) fp32; the reference masks W
with eye(C) so only W[c,c,:,:] survives -> depthwise 3x3 "same" conv.

v3 strategy (per core; data-parallel over the 32 (s,b) samples -> 4/core):
  - bf16 in HBM both directions (host casts x -> bf16, upcasts out -> f32):
    halves HBM traffic vs fp32, DMA floor ~47us/core at ~358GB/s.
  - work tile = (sample, 128-channel block): x0p [128, 66, 64] bf16 with
    zero pad rows 0/65 (kills row-clipping everywhere).
  - PE: the 6 column-shifted taps as diagonal-stationary bf16 matmuls into
    fp32 PSUM (4-bank halves, 512-elem bank chunks, clipped col views).
  - DVE/ACT: the 3 dw=0 taps: per-tap products (tensor_scalar 4x / ACT mul),
    pre-summed into one S2 tile with full-tile 2x adds OFF the critical
    path; after the ACT psum->bf16 evict only ONE 2x add per half remains
    before the store.
  - head: tile-0 x loaded in row-chunks before the (split) weight loads so
    the first matmul issues ~3us in; tail: last tile evict/add/store at
    2-chunk granularity.
"""

import os
from contextlib import ExitStack

import numpy as np
import ml_dtypes

import concourse.bass as bass
import concourse.tile as tile
from concourse import bacc, mybir
from concourse.bass_utils import run_bass_kernel_spmd

S, B, C, H, W_SP = 2, 16, 256, 64, 64
N_CORES = 8
N_SAMPLES = S * B                      # 32
SPC = N_SAMPLES // N_CORES             # 4 samples per core
NBLK = C // 128                        # 2 channel blocks
N_TILES = SPC * NBLK                   # 8 work tiles per core
HP = H + 2                             # 66: zero row, 64 data rows, zero row
ROWS_PER_CHUNK = 8                     # 512 fp32 = one PSUM bank
HALF_CHUNKS = 4                        # chunks per half tile (4 banks)
HALF_ROWS = HALF_CHUNKS * ROWS_PER_CHUNK  # 32
HSPLIT = HALF_ROWS + 2                 # x data rows in first in-DMA half

TAPS = [(0, 0), (-1, -1), (-1, 0), (-1, 1), (0, -1), (0, 1), (1, -1), (1, 0), (1, 1)]
DVE_TAPS = [(0, 0), (-1, 0), (1, 0)]                       # dw=0: DVE/ACT side
PE_TAPS = [(-1, -1), (-1, 1), (0, -1), (0, 1), (1, -1), (1, 1)]  # PE side

# per-tile count of dw0 multiplies routed to ACT (balance ACT vs DVE)
ACT_MULS_LIST = tuple(
    int(v) for v in os.environ.get("KERNEL_ACT_MULS_LIST",
                                   "1,1,1,1,1,1,1,1").split(","))
# tiles where one column tap (0,1) moves from PE to the S2 side via a
# column-shifted padded copy x1p (built on ACT; ACT is 1x anyway)
X1P_TILES = frozenset(
    int(v) for v in os.environ.get("KERNEL_X1P_TILES", "").split(",")
    if v != "")
STAGGER = float(os.environ.get("KERNEL_STAGGER", "0.007"))
WARMUP_MMS = int(os.environ.get("KERNEL_WARMUP_MMS", "0"))
MOVED_TAP = (0, 1)
WPAD = W_SP + 2

F32 = mybir.dt.float32
BF16 = mybir.dt.bfloat16


def _slot(cb, tap):
    return cb * 9 + TAPS.index(tap)


def _load_x(nc, tc, x0_pool, g, x_d):
    """Padded x tile; tile 0 loads in finer chunks to cut the pipeline head."""
    x0p = x0_pool.tile([128, HP, W_SP], BF16, tag="x0p")
    splits = (18, HSPLIT, H) if g == 0 else ((HSPLIT, H) if g == 1 else (H,))
    with tc.tile_wait_until(g * STAGGER):
        nc.vector.memset(x0p[:, 0:1, :], 0.0)
        nc.vector.memset(x0p[:, HP - 1:HP, :], 0.0)
        r0 = 0
        for r1 in splits:
            nc.sync.dma_start(x0p[:, 1 + r0:1 + r1, :],
                              x_d[g * 128:(g + 1) * 128, r0:r1, :])
            r0 = r1
    return x0p


def _emit_dw0_sum(nc, g, x0p, wv_sb, tmp_pool, s2_pool, x1p_pool):
    """S2 = sum of the dw=0 tap products (plus the moved column tap on
    X1P_TILES), aligned to output rows 0..63.

    All full-tile ops off the evict->store critical path; the adds' shifted
    read views keep even element offsets (4B-aligned) so tensor_tensor
    stays in 2x mode.
    """
    cb = g % NBLK
    act_muls = ACT_MULS_LIST[g]
    tmps = []
    for j, (dh, _) in enumerate(DVE_TAPS):
        s = _slot(cb, (dh, 0))
        wv = wv_sb[:, s:s + 1]
        tmp = tmp_pool.tile([128, HP, W_SP], BF16, tag="tmp")
        if j < act_muls:
            nc.scalar.mul(tmp[:], x0p[:], wv)
        else:
            nc.vector.tensor_scalar(tmp[:], x0p[:], wv, None,
                                    mybir.AluOpType.mult)
        tmps.append((dh, 0, W_SP, tmp))
    if g in X1P_TILES:
        # x1p: data at cols 1..64 of a 66-wide padded tile; built on ACT
        # (1x regardless, dodging the odd-offset DVE penalty)
        x1p = x1p_pool.tile([128, HP, WPAD], BF16, tag="x1p")
        nc.vector.memset(x1p[:, :, 0:1], 0.0)
        nc.vector.memset(x1p[:, :, WPAD - 1:WPAD], 0.0)
        nc.scalar.copy(x1p[:, :, 1:1 + W_SP], x0p[:])
        dh, dw = MOVED_TAP
        s = _slot(cb, MOVED_TAP)
        tmp = x1p_pool.tile([128, HP, WPAD], BF16, tag="tmp1p")
        nc.vector.tensor_scalar(tmp[:], x1p[:], wv_sb[:, s:s + 1], None,
                                mybir.AluOpType.mult)
        tmps.append((dh, 1 + dw, WPAD, tmp))
    s2 = s2_pool.tile([128, H, W_SP], BF16, tag="s2")
    first = True
    prev = None
    for dh, c0, wid, tmp in tmps:
        view = tmp[:, 1 + dh:1 + dh + H, c0:c0 + W_SP]
        if first:
            prev = view
            first = False
            continue
        nc.vector.tensor_tensor(s2[:], prev, view, op=mybir.AluOpType.add)
        prev = s2[:]
    return s2


def _emit_pe_half(nc, g, half, x0p, wd_sb, psum):
    """6 column-shifted taps into psum for rows [32*half, 32*half+32).

    Padded rows -> no row clipping. Clipped col views: start=True on tap 0
    clears the bank; each element's first writer overwrites, later ones
    accumulate (order independent).
    """
    cb = g % NBLK
    taps = [t for t in PE_TAPS if not (g in X1P_TILES and t == MOVED_TAP)]
    # chunk-major: each PSUM bank's accumulation group is contiguous in
    # time, and early chunks only need the first few input rows -> the
    # first matmuls start as soon as the first small load piece lands
    for q in range(HALF_CHUNKS):
        for i, (dh, dw) in enumerate(taps):
            s = _slot(cb, (dh, dw))
            lhsT = wd_sb[:, s * 128:(s + 1) * 128]
            co0 = max(0, -dw)
            co1 = W_SP - max(0, dw)
            r = 1 + dh + half * HALF_ROWS + q * ROWS_PER_CHUNK
            rhs = x0p[:, r:r + ROWS_PER_CHUNK, co0 + dw:co1 + dw]
            nc.tensor.matmul(psum[:, q, :, co0:co1], lhsT, rhs,
                             start=(i == 0), stop=(i == len(taps) - 1))


def _emit_tile(nc, tc, g, x_d, out_d, wd_sb, wv_sb, pools, x0p=None):
    x0_pool, tmp_pool, s2_pool, osb_pool, psum_pool, x1p_pool = pools
    if x0p is None:
        x0p = _load_x(nc, tc, x0_pool, g, x_d)
    s2 = _emit_dw0_sum(nc, g, x0p, wv_sb, tmp_pool, s2_pool, x1p_pool)
    osb = osb_pool.tile([128, H, W_SP], BF16, tag="osb")
    for half in range(2):
        psum = psum_pool.tile([128, HALF_CHUNKS, ROWS_PER_CHUNK, W_SP], F32,
                              tag="psum")
        _emit_pe_half(nc, g, half, x0p, wd_sb, psum)
        # evict + single add + store; finer grain near the end to cut the
        # pipeline tail
        if g == N_TILES - 1 and half == 1:
            pieces = 4
        elif g >= N_TILES - 2:
            pieces = 2
        else:
            pieces = 1
        rows_pp = HALF_ROWS // pieces
        whole_store = g < N_TILES - 2
        for p in range(pieces):
            r0 = half * HALF_ROWS + p * rows_pp
            ov = osb[:, r0:r0 + rows_pp, :]
            nc.scalar.copy(ov, psum[:, p * (HALF_CHUNKS // pieces):
                                    (p + 1) * (HALF_CHUNKS // pieces), :, :])
            nc.vector.tensor_tensor(ov, ov, s2[:, r0:r0 + rows_pp, :],
                                    op=mybir.AluOpType.add)
            if not whole_store:
                nc.sync.dma_start(
                    out_d[g * 128:(g + 1) * 128, r0:r0 + rows_pp, :], ov)
        if whole_store and half == 1:
            nc.sync.dma_start(out_d[g * 128:(g + 1) * 128, :, :], osb[:])


def _build_program():
    nc = bacc.Bacc("TRN2", target_bir_lowering=False, debug=False)
    x_d = nc.dram_tensor("x", [SPC * C, H, W_SP], BF16, kind="ExternalInput").ap()
    wd_d = nc.dram_tensor("wd", [128, NBLK * 9 * 128], BF16, kind="ExternalInput").ap()
    wv_d = nc.dram_tensor("wv", [128, NBLK * 9], F32, kind="ExternalInput").ap()
    out_d = nc.dram_tensor("out", [SPC * C, H, W_SP], BF16, kind="ExternalOutput").ap()

    # cb=0 PE-tap weight slots first so the first matmuls unblock early
    wd_head = sorted(_slot(0, t) for t in PE_TAPS)

    with tile.TileContext(nc) as tc:
        with ExitStack() as ctx:
            const_pool = ctx.enter_context(tc.tile_pool(name="const", bufs=1))
            wd_sb = const_pool.tile([128, NBLK * 9 * 128], BF16)
            wv_sb = const_pool.tile([128, NBLK * 9], F32)
            done = set()
            for j, s in enumerate(wd_head):
                eng = nc.sync if j == 0 else nc.gpsimd
                eng.dma_start(wd_sb[:, s * 128:(s + 1) * 128],
                              wd_d[:, s * 128:(s + 1) * 128])
                done.add(s)
            nc.gpsimd.dma_start(wv_sb[:], wv_d[:])
            rest = [s for s in range(NBLK * 9) if s not in done]
            # remaining slots are contiguous runs; batch them
            r0 = 0
            while r0 < len(rest):
                r1 = r0
                while r1 + 1 < len(rest) and rest[r1 + 1] == rest[r1] + 1:
                    r1 += 1
                a, b = rest[r0], rest[r1] + 1
                nc.gpsimd.dma_start(wd_sb[:, a * 128:b * 128],
                                    wd_d[:, a * 128:b * 128])
                r0 = r1 + 1

            psum_pool = ctx.enter_context(tc.tile_pool(name="psum", bufs=2, space="PSUM"))
            if WARMUP_MMS:
                wlhs = const_pool.tile([128, 128], BF16)
                wrhs = const_pool.tile([128, ROWS_PER_CHUNK, W_SP], BF16)
                nc.vector.memset(wlhs[:], 0.0)
                nc.vector.memset(wrhs[:], 0.0)
                wp = psum_pool.tile([128, HALF_CHUNKS, ROWS_PER_CHUNK, W_SP],
                                    F32, tag="psum")
                for _ in range(WARMUP_MMS):
                    nc.tensor.matmul(wp[:, 0, :, :], wlhs[:], wrhs[:],
                                     start=True, stop=True)
            x0_pool = ctx.enter_context(tc.tile_pool(name="x0", bufs=5))
            tmp_pool = ctx.enter_context(tc.tile_pool(name="tmp", bufs=4))
            s2_pool = ctx.enter_context(tc.tile_pool(name="s2", bufs=3))
            osb_pool = ctx.enter_context(tc.tile_pool(name="osb", bufs=4))
            x1p_pool = ctx.enter_context(tc.tile_pool(name="x1p", bufs=2))
            pools = (x0_pool, tmp_pool, s2_pool, osb_pool, psum_pool, x1p_pool)

            for g in range(N_TILES):
                _emit_tile(nc, tc, g, x_d, out_d, wd_sb, wv_sb, pools)
    nc.compile()
    return nc


_prog_cache = {}


def _get_program():
    key = (ACT_MULS_LIST, X1P_TILES, STAGGER)
    if key not in _prog_cache:
        _prog_cache[key] = _build_program()
    return _prog_cache[key]


def _host_weights(W):
    wdiag = W[np.arange(C), np.arange(C)]          # [256, 3, 3]
    wd_host = np.zeros((128, NBLK * 9, 128), dtype=np.float32)
    wv_host = np.zeros((128, NBLK * 9), dtype=np.float32)
    r = np.arange(128)
    for cb in range(NBLK):
        for t, (dh, dw) in enumerate(TAPS):
            wd_host[r, cb * 9 + t, r] = wdiag[cb * 128 + r, dh + 1, dw + 1]
            wv_host[r, cb * 9 + t] = wdiag[cb * 128 + r, dh + 1, dw + 1]
    return wd_host.reshape(128, NBLK * 9 * 128).astype(ml_dtypes.bfloat16), wv_host


def _in_maps(x, W):
    wd_host, wv_host = _host_weights(np.asarray(W, dtype=np.float32))
    xb = np.asarray(x, dtype=np.float32).astype(ml_dtypes.bfloat16)
    xs = xb.reshape(N_SAMPLES, C, H, W_SP)
    return [
        {
            "x": np.ascontiguousarray(xs[i * SPC:(i + 1) * SPC]).reshape(SPC * C, H, W_SP),
            "wd": wd_host,
            "wv": wv_host,
        }
        for i in range(N_CORES)
    ]


def kernel(x: np.ndarray, W: np.ndarray) -> np.ndarray:
    x = np.ascontiguousarray(x, dtype=np.float32)
    W = np.ascontiguousarray(W, dtype=np.float32)
    assert x.shape == (S, B, C, H, W_SP)
    assert W.shape == (C, C, 3, 3)

    nc = _get_program()
    res = run_bass_kernel_spmd(nc, _in_maps(x, W), core_ids=list(range(N_CORES)))
    out = np.concatenate(
        [res.results[i]["out"].reshape(SPC, C, H, W_SP).astype(np.float32)
         for i in range(N_CORES)],
        axis=0,
    )
    return out.reshape(S, B, C, H, W_SP)


# revision 18
# speedup vs baseline: 1.0117x; 1.0117x over previous
"""Depthwise-masked 3x3 conv (eye-masked dense conv) on 8 TRN2 NeuronCores.

Problem: x (2,16,256,64,64) fp32, W (256,256,3,3) fp32; the reference masks W
with eye(C) so only W[c,c,:,:] survives -> depthwise 3x3 "same" conv.

v3 strategy (per core; data-parallel over the 32 (s,b) samples -> 4/core):
  - bf16 in HBM both directions (host casts x -> bf16, upcasts out -> f32):
    halves HBM traffic vs fp32, DMA floor ~47us/core at ~358GB/s.
  - work tile = (sample, 128-channel block): x0p [128, 66, 64] bf16 with
    zero pad rows 0/65 (kills row-clipping everywhere).
  - PE: the 6 column-shifted taps as diagonal-stationary bf16 matmuls into
    fp32 PSUM (4-bank halves, 512-elem bank chunks, clipped col views).
  - DVE/ACT: the 3 dw=0 taps: per-tap products (tensor_scalar 4x / ACT mul),
    pre-summed into one S2 tile with full-tile 2x adds OFF the critical
    path; after the ACT psum->bf16 evict only ONE 2x add per half remains
    before the store.
  - head: tile-0 x loaded in row-chunks before the (split) weight loads so
    the first matmul issues ~3us in; tail: last tile evict/add/store at
    2-chunk granularity.
"""

import os
from contextlib import ExitStack

import numpy as np
import ml_dtypes

import concourse.bass as bass
import concourse.tile as tile
from concourse import bacc, mybir
from concourse.bass_utils import run_bass_kernel_spmd

S, B, C, H, W_SP = 2, 16, 256, 64, 64
N_CORES = 8
N_SAMPLES = S * B                      # 32
SPC = N_SAMPLES // N_CORES             # 4 samples per core
NBLK = C // 128                        # 2 channel blocks
N_TILES = SPC * NBLK                   # 8 work tiles per core
HP = H + 2                             # 66: zero row, 64 data rows, zero row
ROWS_PER_CHUNK = 8                     # 512 fp32 = one PSUM bank
HALF_CHUNKS = 4                        # chunks per half tile (4 banks)
HALF_ROWS = HALF_CHUNKS * ROWS_PER_CHUNK  # 32
HSPLIT = HALF_ROWS + 2                 # x data rows in first in-DMA half

TAPS = [(0, 0), (-1, -1), (-1, 0), (-1, 1), (0, -1), (0, 1), (1, -1), (1, 0), (1, 1)]
DVE_TAPS = [(0, 0), (-1, 0), (1, 0)]                       # dw=0: DVE/ACT side
PE_TAPS = [(-1, -1), (-1, 1), (0, -1), (0, 1), (1, -1), (1, 1)]  # PE side

# per-tile count of dw0 multiplies routed to ACT (balance ACT vs DVE)
ACT_MULS_LIST = tuple(
    int(v) for v in os.environ.get("KERNEL_ACT_MULS_LIST",
                                   "1,1,1,1,1,1,1,1").split(","))
# tiles where one column tap (0,1) moves from PE to the S2 side via a
# column-shifted padded copy x1p (built on ACT; ACT is 1x anyway)
X1P_TILES = frozenset(
    int(v) for v in os.environ.get("KERNEL_X1P_TILES", "").split(",")
    if v != "")
STAGGER = float(os.environ.get("KERNEL_STAGGER", "0.007"))
WARMUP_MMS = int(os.environ.get("KERNEL_WARMUP_MMS", "0"))
MOVED_TAP = (0, 1)
WPAD = W_SP + 2

F32 = mybir.dt.float32
BF16 = mybir.dt.bfloat16


def _slot(cb, tap):
    return cb * 9 + TAPS.index(tap)


def _load_x(nc, tc, x0_pool, g, x_d):
    """Padded x tile; tile 0 loads in finer chunks to cut the pipeline head."""
    x0p = x0_pool.tile([128, HP, W_SP], BF16, tag="x0p")
    splits = (10, 18, 26, HSPLIT, H) if g == 0 else ((HSPLIT, H) if g == 1 else (H,))
    with tc.tile_wait_until(g * STAGGER):
        nc.vector.memset(x0p[:, 0:1, :], 0.0)
        nc.vector.memset(x0p[:, HP - 1:HP, :], 0.0)
        r0 = 0
        for r1 in splits:
            nc.sync.dma_start(x0p[:, 1 + r0:1 + r1, :],
                              x_d[g * 128:(g + 1) * 128, r0:r1, :])
            r0 = r1
    return x0p


def _emit_dw0_sum(nc, g, x0p, wv_sb, tmp_pool, s2_pool, x1p_pool):
    """S2 = sum of the dw=0 tap products (plus the moved column tap on
    X1P_TILES), aligned to output rows 0..63.

    All full-tile ops off the evict->store critical path; the adds' shifted
    read views keep even element offsets (4B-aligned) so tensor_tensor
    stays in 2x mode.
    """
    cb = g % NBLK
    act_muls = ACT_MULS_LIST[g]
    tmps = []
    for j, (dh, _) in enumerate(DVE_TAPS):
        s = _slot(cb, (dh, 0))
        wv = wv_sb[:, s:s + 1]
        tmp = tmp_pool.tile([128, HP, W_SP], BF16, tag="tmp")
        if j < act_muls:
            nc.scalar.mul(tmp[:], x0p[:], wv)
        else:
            nc.vector.tensor_scalar(tmp[:], x0p[:], wv, None,
                                    mybir.AluOpType.mult)
        tmps.append((dh, 0, W_SP, tmp))
    if g in X1P_TILES:
        # x1p: data at cols 1..64 of a 66-wide padded tile; built on ACT
        # (1x regardless, dodging the odd-offset DVE penalty)
        x1p = x1p_pool.tile([128, HP, WPAD], BF16, tag="x1p")
        nc.vector.memset(x1p[:, :, 0:1], 0.0)
        nc.vector.memset(x1p[:, :, WPAD - 1:WPAD], 0.0)
        nc.scalar.copy(x1p[:, :, 1:1 + W_SP], x0p[:])
        dh, dw = MOVED_TAP
        s = _slot(cb, MOVED_TAP)
        tmp = x1p_pool.tile([128, HP, WPAD], BF16, tag="tmp1p")
        nc.vector.tensor_scalar(tmp[:], x1p[:], wv_sb[:, s:s + 1], None,
                                mybir.AluOpType.mult)
        tmps.append((dh, 1 + dw, WPAD, tmp))
    s2 = s2_pool.tile([128, H, W_SP], BF16, tag="s2")
    first = True
    prev = None
    for dh, c0, wid, tmp in tmps:
        view = tmp[:, 1 + dh:1 + dh + H, c0:c0 + W_SP]
        if first:
            prev = view
            first = False
            continue
        nc.vector.tensor_tensor(s2[:], prev, view, op=mybir.AluOpType.add)
        prev = s2[:]
    return s2


def _emit_pe_half(nc, g, half, x0p, wd_sb, psum):
    """6 column-shifted taps into psum for rows [32*half, 32*half+32).

    Padded rows -> no row clipping. Clipped col views: start=True on tap 0
    clears the bank; each element's first writer overwrites, later ones
    accumulate (order independent).
    """
    cb = g % NBLK
    taps = [t for t in PE_TAPS if not (g in X1P_TILES and t == MOVED_TAP)]
    # chunk-major: each PSUM bank's accumulation group is contiguous in
    # time, and early chunks only need the first few input rows -> the
    # first matmuls start as soon as the first small load piece lands
    for q in range(HALF_CHUNKS):
        for i, (dh, dw) in enumerate(taps):
            s = _slot(cb, (dh, dw))
            lhsT = wd_sb[:, s * 128:(s + 1) * 128]
            co0 = max(0, -dw)
            co1 = W_SP - max(0, dw)
            r = 1 + dh + half * HALF_ROWS + q * ROWS_PER_CHUNK
            rhs = x0p[:, r:r + ROWS_PER_CHUNK, co0 + dw:co1 + dw]
            nc.tensor.matmul(psum[:, q, :, co0:co1], lhsT, rhs,
                             start=(i == 0), stop=(i == len(taps) - 1))


def _emit_tile(nc, tc, g, x_d, out_d, wd_sb, wv_sb, pools, x0p=None):
    x0_pool, tmp_pool, s2_pool, osb_pool, psum_pool, x1p_pool = pools
    if x0p is None:
        x0p = _load_x(nc, tc, x0_pool, g, x_d)
    s2 = _emit_dw0_sum(nc, g, x0p, wv_sb, tmp_pool, s2_pool, x1p_pool)
    osb = osb_pool.tile([128, H, W_SP], BF16, tag="osb")
    for half in range(2):
        psum = psum_pool.tile([128, HALF_CHUNKS, ROWS_PER_CHUNK, W_SP], F32,
                              tag="psum")
        _emit_pe_half(nc, g, half, x0p, wd_sb, psum)
        # evict + single add + store; finer grain near the end to cut the
        # pipeline tail
        if g == N_TILES - 1 and half == 1:
            pieces = 4
        elif g >= N_TILES - 2:
            pieces = 2
        else:
            pieces = 1
        rows_pp = HALF_ROWS // pieces
        whole_store = g < N_TILES - 2
        for p in range(pieces):
            r0 = half * HALF_ROWS + p * rows_pp
            ov = osb[:, r0:r0 + rows_pp, :]
            nc.scalar.copy(ov, psum[:, p * (HALF_CHUNKS // pieces):
                                    (p + 1) * (HALF_CHUNKS // pieces), :, :])
            nc.vector.tensor_tensor(ov, ov, s2[:, r0:r0 + rows_pp, :],
                                    op=mybir.AluOpType.add)
            if not whole_store:
                nc.sync.dma_start(
                    out_d[g * 128:(g + 1) * 128, r0:r0 + rows_pp, :], ov)
        if whole_store and half == 1:
            nc.sync.dma_start(out_d[g * 128:(g + 1) * 128, :, :], osb[:])


def _build_program():
    nc = bacc.Bacc("TRN2", target_bir_lowering=False, debug=False)
    x_d = nc.dram_tensor("x", [SPC * C, H, W_SP], BF16, kind="ExternalInput").ap()
    wd_d = nc.dram_tensor("wd", [128, NBLK * 9 * 128], BF16, kind="ExternalInput").ap()
    wv_d = nc.dram_tensor("wv", [128, NBLK * 9], F32, kind="ExternalInput").ap()
    out_d = nc.dram_tensor("out", [SPC * C, H, W_SP], BF16, kind="ExternalOutput").ap()

    # cb=0 PE-tap weight slots first so the first matmuls unblock early
    wd_head = sorted(_slot(0, t) for t in PE_TAPS)

    with tile.TileContext(nc) as tc:
        with ExitStack() as ctx:
            const_pool = ctx.enter_context(tc.tile_pool(name="const", bufs=1))
            wd_sb = const_pool.tile([128, NBLK * 9 * 128], BF16)
            wv_sb = const_pool.tile([128, NBLK * 9], F32)
            done = set()
            for j, s in enumerate(wd_head):
                eng = nc.sync if j == 0 else nc.gpsimd
                eng.dma_start(wd_sb[:, s * 128:(s + 1) * 128],
                              wd_d[:, s * 128:(s + 1) * 128])
                done.add(s)
            nc.gpsimd.dma_start(wv_sb[:], wv_d[:])
            rest = [s for s in range(NBLK * 9) if s not in done]
            # remaining slots are contiguous runs; batch them
            r0 = 0
            while r0 < len(rest):
                r1 = r0
                while r1 + 1 < len(rest) and rest[r1 + 1] == rest[r1] + 1:
                    r1 += 1
                a, b = rest[r0], rest[r1] + 1
                nc.gpsimd.dma_start(wd_sb[:, a * 128:b * 128],
                                    wd_d[:, a * 128:b * 128])
                r0 = r1 + 1

            psum_pool = ctx.enter_context(tc.tile_pool(name="psum", bufs=2, space="PSUM"))
            if WARMUP_MMS:
                wlhs = const_pool.tile([128, 128], BF16)
                wrhs = const_pool.tile([128, ROWS_PER_CHUNK, W_SP], BF16)
                nc.vector.memset(wlhs[:], 0.0)
                nc.vector.memset(wrhs[:], 0.0)
                wp = psum_pool.tile([128, HALF_CHUNKS, ROWS_PER_CHUNK, W_SP],
                                    F32, tag="psum")
                for _ in range(WARMUP_MMS):
                    nc.tensor.matmul(wp[:, 0, :, :], wlhs[:], wrhs[:],
                                     start=True, stop=True)
            x0_pool = ctx.enter_context(tc.tile_pool(name="x0", bufs=5))
            tmp_pool = ctx.enter_context(tc.tile_pool(name="tmp", bufs=4))
            s2_pool = ctx.enter_context(tc.tile_pool(name="s2", bufs=3))
            osb_pool = ctx.enter_context(tc.tile_pool(name="osb", bufs=4))
            x1p_pool = ctx.enter_context(tc.tile_pool(name="x1p", bufs=2))
            pools = (x0_pool, tmp_pool, s2_pool, osb_pool, psum_pool, x1p_pool)

            for g in range(N_TILES):
                _emit_tile(nc, tc, g, x_d, out_d, wd_sb, wv_sb, pools)
    nc.compile()
    return nc


_prog_cache = {}


def _get_program():
    key = (ACT_MULS_LIST, X1P_TILES, STAGGER)
    if key not in _prog_cache:
        _prog_cache[key] = _build_program()
    return _prog_cache[key]


def _host_weights(W):
    wdiag = W[np.arange(C), np.arange(C)]          # [256, 3, 3]
    wd_host = np.zeros((128, NBLK * 9, 128), dtype=np.float32)
    wv_host = np.zeros((128, NBLK * 9), dtype=np.float32)
    r = np.arange(128)
    for cb in range(NBLK):
        for t, (dh, dw) in enumerate(TAPS):
            wd_host[r, cb * 9 + t, r] = wdiag[cb * 128 + r, dh + 1, dw + 1]
            wv_host[r, cb * 9 + t] = wdiag[cb * 128 + r, dh + 1, dw + 1]
    return wd_host.reshape(128, NBLK * 9 * 128).astype(ml_dtypes.bfloat16), wv_host


def _in_maps(x, W):
    wd_host, wv_host = _host_weights(np.asarray(W, dtype=np.float32))
    xb = np.asarray(x, dtype=np.float32).astype(ml_dtypes.bfloat16)
    xs = xb.reshape(N_SAMPLES, C, H, W_SP)
    return [
        {
            "x": np.ascontiguousarray(xs[i * SPC:(i + 1) * SPC]).reshape(SPC * C, H, W_SP),
            "wd": wd_host,
            "wv": wv_host,
        }
        for i in range(N_CORES)
    ]


def kernel(x: np.ndarray, W: np.ndarray) -> np.ndarray:
    x = np.ascontiguousarray(x, dtype=np.float32)
    W = np.ascontiguousarray(W, dtype=np.float32)
    assert x.shape == (S, B, C, H, W_SP)
    assert W.shape == (C, C, 3, 3)

    nc = _get_program()
    res = run_bass_kernel_spmd(nc, _in_maps(x, W), core_ids=list(range(N_CORES)))
    out = np.concatenate(
        [res.results[i]["out"].reshape(SPC, C, H, W_SP).astype(np.float32)
         for i in range(N_CORES)],
        axis=0,
    )
    return out.reshape(S, B, C, H, W_SP)


# revision 19
# speedup vs baseline: 1.0228x; 1.0110x over previous
"""Depthwise-masked 3x3 conv (eye-masked dense conv) on 8 TRN2 NeuronCores.

Problem: x (2,16,256,64,64) fp32, W (256,256,3,3) fp32; the reference masks W
with eye(C) so only W[c,c,:,:] survives -> depthwise 3x3 "same" conv.

v3 strategy (per core; data-parallel over the 32 (s,b) samples -> 4/core):
  - bf16 in HBM both directions (host casts x -> bf16, upcasts out -> f32):
    halves HBM traffic vs fp32, DMA floor ~47us/core at ~358GB/s.
  - work tile = (sample, 128-channel block): x0p [128, 66, 64] bf16 with
    zero pad rows 0/65 (kills row-clipping everywhere).
  - PE: the 6 column-shifted taps as diagonal-stationary bf16 matmuls into
    fp32 PSUM (4-bank halves, 512-elem bank chunks, clipped col views).
  - DVE/ACT: the 3 dw=0 taps: per-tap products (tensor_scalar 4x / ACT mul),
    pre-summed into one S2 tile with full-tile 2x adds OFF the critical
    path; after the ACT psum->bf16 evict only ONE 2x add per half remains
    before the store.
  - head: tile-0 x loaded in row-chunks before the (split) weight loads so
    the first matmul issues ~3us in; tail: last tile evict/add/store at
    2-chunk granularity.
"""

import os
from contextlib import ExitStack

import numpy as np
import ml_dtypes

import concourse.bass as bass
import concourse.tile as tile
from concourse import bacc, mybir
from concourse.bass_utils import run_bass_kernel_spmd

S, B, C, H, W_SP = 2, 16, 256, 64, 64
N_CORES = 8
N_SAMPLES = S * B                      # 32
SPC = N_SAMPLES // N_CORES             # 4 samples per core
NBLK = C // 128                        # 2 channel blocks
N_TILES = SPC * NBLK                   # 8 work tiles per core
HP = H + 2                             # 66: zero row, 64 data rows, zero row
ROWS_PER_CHUNK = 8                     # 512 fp32 = one PSUM bank
HALF_CHUNKS = 4                        # chunks per half tile (4 banks)
HALF_ROWS = HALF_CHUNKS * ROWS_PER_CHUNK  # 32
HSPLIT = HALF_ROWS + 2                 # x data rows in first in-DMA half

TAPS = [(0, 0), (-1, -1), (-1, 0), (-1, 1), (0, -1), (0, 1), (1, -1), (1, 0), (1, 1)]
DVE_TAPS = [(0, 0), (-1, 0), (1, 0)]                       # dw=0: DVE/ACT side
PE_TAPS = [(-1, -1), (-1, 1), (0, -1), (0, 1), (1, -1), (1, 1)]  # PE side

# per-tile count of dw0 multiplies routed to ACT (balance ACT vs DVE)
ACT_MULS_LIST = tuple(
    int(v) for v in os.environ.get("KERNEL_ACT_MULS_LIST",
                                   "1,1,1,1,1,1,1,1").split(","))
# tiles where one column tap (0,1) moves from PE to the S2 side via a
# column-shifted padded copy x1p (built on ACT; ACT is 1x anyway)
X1P_TILES = frozenset(
    int(v) for v in os.environ.get("KERNEL_X1P_TILES", "").split(",")
    if v != "")
STAGGER = float(os.environ.get("KERNEL_STAGGER", "0.007"))
WARMUP_MMS = int(os.environ.get("KERNEL_WARMUP_MMS", "0"))
MOVED_TAP = (0, 1)
WPAD = W_SP + 2

F32 = mybir.dt.float32
BF16 = mybir.dt.bfloat16


def _slot(cb, tap):
    return cb * 9 + TAPS.index(tap)


def _load_x(nc, tc, x0_pool, g, x_d):
    """Padded x tile; tile 0 loads in finer chunks to cut the pipeline head."""
    x0p = x0_pool.tile([128, HP, W_SP], BF16, tag="x0p")
    splits = (10, HSPLIT, H) if g == 0 else ((HSPLIT, H) if g == 1 else (H,))
    with tc.tile_wait_until(g * STAGGER):
        nc.vector.memset(x0p[:, 0:1, :], 0.0)
        nc.vector.memset(x0p[:, HP - 1:HP, :], 0.0)
        r0 = 0
        for r1 in splits:
            nc.sync.dma_start(x0p[:, 1 + r0:1 + r1, :],
                              x_d[g * 128:(g + 1) * 128, r0:r1, :])
            r0 = r1
    return x0p


def _emit_dw0_sum(nc, g, x0p, wv_sb, tmp_pool, s2_pool, x1p_pool):
    """S2 = sum of the dw=0 tap products (plus the moved column tap on
    X1P_TILES), aligned to output rows 0..63.

    All full-tile ops off the evict->store critical path; the adds' shifted
    read views keep even element offsets (4B-aligned) so tensor_tensor
    stays in 2x mode.
    """
    cb = g % NBLK
    act_muls = ACT_MULS_LIST[g]
    tmps = []
    for j, (dh, _) in enumerate(DVE_TAPS):
        s = _slot(cb, (dh, 0))
        wv = wv_sb[:, s:s + 1]
        tmp = tmp_pool.tile([128, HP, W_SP], BF16, tag="tmp")
        if j < act_muls:
            nc.scalar.mul(tmp[:], x0p[:], wv)
        else:
            nc.vector.tensor_scalar(tmp[:], x0p[:], wv, None,
                                    mybir.AluOpType.mult)
        tmps.append((dh, 0, W_SP, tmp))
    if g in X1P_TILES:
        # x1p: data at cols 1..64 of a 66-wide padded tile; built on ACT
        # (1x regardless, dodging the odd-offset DVE penalty)
        x1p = x1p_pool.tile([128, HP, WPAD], BF16, tag="x1p")
        nc.vector.memset(x1p[:, :, 0:1], 0.0)
        nc.vector.memset(x1p[:, :, WPAD - 1:WPAD], 0.0)
        nc.scalar.copy(x1p[:, :, 1:1 + W_SP], x0p[:])
        dh, dw = MOVED_TAP
        s = _slot(cb, MOVED_TAP)
        tmp = x1p_pool.tile([128, HP, WPAD], BF16, tag="tmp1p")
        nc.vector.tensor_scalar(tmp[:], x1p[:], wv_sb[:, s:s + 1], None,
                                mybir.AluOpType.mult)
        tmps.append((dh, 1 + dw, WPAD, tmp))
    s2 = s2_pool.tile([128, H, W_SP], BF16, tag="s2")
    first = True
    prev = None
    for dh, c0, wid, tmp in tmps:
        view = tmp[:, 1 + dh:1 + dh + H, c0:c0 + W_SP]
        if first:
            prev = view
            first = False
            continue
        nc.vector.tensor_tensor(s2[:], prev, view, op=mybir.AluOpType.add)
        prev = s2[:]
    return s2


def _emit_pe_half(nc, g, half, x0p, wd_sb, psum):
    """6 column-shifted taps into psum for rows [32*half, 32*half+32).

    Padded rows -> no row clipping. Clipped col views: start=True on tap 0
    clears the bank; each element's first writer overwrites, later ones
    accumulate (order independent).
    """
    cb = g % NBLK
    taps = [t for t in PE_TAPS if not (g in X1P_TILES and t == MOVED_TAP)]
    for i, (dh, dw) in enumerate(taps):
        s = _slot(cb, (dh, dw))
        lhsT = wd_sb[:, s * 128:(s + 1) * 128]
        co0 = max(0, -dw)
        co1 = W_SP - max(0, dw)
        for q in range(HALF_CHUNKS):
            r = 1 + dh + half * HALF_ROWS + q * ROWS_PER_CHUNK
            rhs = x0p[:, r:r + ROWS_PER_CHUNK, co0 + dw:co1 + dw]
            nc.tensor.matmul(psum[:, q, :, co0:co1], lhsT, rhs,
                             start=(i == 0), stop=(i == len(taps) - 1))


def _emit_tile(nc, tc, g, x_d, out_d, wd_sb, wv_sb, pools, x0p=None):
    x0_pool, tmp_pool, s2_pool, osb_pool, psum_pool, x1p_pool = pools
    if x0p is None:
        x0p = _load_x(nc, tc, x0_pool, g, x_d)
    s2 = _emit_dw0_sum(nc, g, x0p, wv_sb, tmp_pool, s2_pool, x1p_pool)
    osb = osb_pool.tile([128, H, W_SP], BF16, tag="osb")
    for half in range(2):
        psum = psum_pool.tile([128, HALF_CHUNKS, ROWS_PER_CHUNK, W_SP], F32,
                              tag="psum")
        _emit_pe_half(nc, g, half, x0p, wd_sb, psum)
        # evict + single add + store; finer grain near the end to cut the
        # pipeline tail
        if g == N_TILES - 1 and half == 1:
            pieces = 4
        elif g >= N_TILES - 2:
            pieces = 2
        else:
            pieces = 1
        rows_pp = HALF_ROWS // pieces
        whole_store = g < N_TILES - 2
        for p in range(pieces):
            r0 = half * HALF_ROWS + p * rows_pp
            ov = osb[:, r0:r0 + rows_pp, :]
            nc.scalar.copy(ov, psum[:, p * (HALF_CHUNKS // pieces):
                                    (p + 1) * (HALF_CHUNKS // pieces), :, :])
            nc.vector.tensor_tensor(ov, ov, s2[:, r0:r0 + rows_pp, :],
                                    op=mybir.AluOpType.add)
            if not whole_store:
                nc.sync.dma_start(
                    out_d[g * 128:(g + 1) * 128, r0:r0 + rows_pp, :], ov)
        if whole_store and half == 1:
            nc.sync.dma_start(out_d[g * 128:(g + 1) * 128, :, :], osb[:])


def _build_program():
    nc = bacc.Bacc("TRN2", target_bir_lowering=False, debug=False)
    x_d = nc.dram_tensor("x", [SPC * C, H, W_SP], BF16, kind="ExternalInput").ap()
    wd_d = nc.dram_tensor("wd", [128, NBLK * 9 * 128], BF16, kind="ExternalInput").ap()
    wv_d = nc.dram_tensor("wv", [128, NBLK * 9], F32, kind="ExternalInput").ap()
    out_d = nc.dram_tensor("out", [SPC * C, H, W_SP], BF16, kind="ExternalOutput").ap()

    # cb=0 PE-tap weight slots first so the first matmuls unblock early
    wd_head = sorted(_slot(0, t) for t in PE_TAPS)

    with tile.TileContext(nc) as tc:
        with ExitStack() as ctx:
            const_pool = ctx.enter_context(tc.tile_pool(name="const", bufs=1))
            wd_sb = const_pool.tile([128, NBLK * 9 * 128], BF16)
            wv_sb = const_pool.tile([128, NBLK * 9], F32)
            done = set()
            for j, s in enumerate(wd_head):
                eng = nc.sync if j == 0 else nc.gpsimd
                eng.dma_start(wd_sb[:, s * 128:(s + 1) * 128],
                              wd_d[:, s * 128:(s + 1) * 128])
                done.add(s)
            nc.gpsimd.dma_start(wv_sb[:], wv_d[:])
            rest = [s for s in range(NBLK * 9) if s not in done]
            # remaining slots are contiguous runs; batch them
            r0 = 0
            while r0 < len(rest):
                r1 = r0
                while r1 + 1 < len(rest) and rest[r1 + 1] == rest[r1] + 1:
                    r1 += 1
                a, b = rest[r0], rest[r1] + 1
                nc.gpsimd.dma_start(wd_sb[:, a * 128:b * 128],
                                    wd_d[:, a * 128:b * 128])
                r0 = r1 + 1

            psum_pool = ctx.enter_context(tc.tile_pool(name="psum", bufs=2, space="PSUM"))
            if WARMUP_MMS:
                wlhs = const_pool.tile([128, 128], BF16)
                wrhs = const_pool.tile([128, ROWS_PER_CHUNK, W_SP], BF16)
                nc.vector.memset(wlhs[:], 0.0)
                nc.vector.memset(wrhs[:], 0.0)
                wp = psum_pool.tile([128, HALF_CHUNKS, ROWS_PER_CHUNK, W_SP],
                                    F32, tag="psum")
                for _ in range(WARMUP_MMS):
                    nc.tensor.matmul(wp[:, 0, :, :], wlhs[:], wrhs[:],
                                     start=True, stop=True)
            x0_pool = ctx.enter_context(tc.tile_pool(name="x0", bufs=5))
            tmp_pool = ctx.enter_context(tc.tile_pool(name="tmp", bufs=4))
            s2_pool = ctx.enter_context(tc.tile_pool(name="s2", bufs=3))
            osb_pool = ctx.enter_context(tc.tile_pool(name="osb", bufs=4))
            x1p_pool = ctx.enter_context(tc.tile_pool(name="x1p", bufs=2))
            pools = (x0_pool, tmp_pool, s2_pool, osb_pool, psum_pool, x1p_pool)

            for g in range(N_TILES):
                _emit_tile(nc, tc, g, x_d, out_d, wd_sb, wv_sb, pools)
    nc.compile()
    return nc


_prog_cache = {}


def _get_program():
    key = (ACT_MULS_LIST, X1P_TILES, STAGGER)
    if key not in _prog_cache:
        _prog_cache[key] = _build_program()
    return _prog_cache[key]


def _host_weights(W):
    wdiag = W[np.arange(C), np.arange(C)]          # [256, 3, 3]
    wd_host = np.zeros((128, NBLK * 9, 128), dtype=np.float32)
    wv_host = np.zeros((128, NBLK * 9), dtype=np.float32)
    r = np.arange(128)
    for cb in range(NBLK):
        for t, (dh, dw) in enumerate(TAPS):
            wd_host[r, cb * 9 + t, r] = wdiag[cb * 128 + r, dh + 1, dw + 1]
            wv_host[r, cb * 9 + t] = wdiag[cb * 128 + r, dh + 1, dw + 1]
    return wd_host.reshape(128, NBLK * 9 * 128).astype(ml_dtypes.bfloat16), wv_host


def _in_maps(x, W):
    wd_host, wv_host = _host_weights(np.asarray(W, dtype=np.float32))
    xb = np.asarray(x, dtype=np.float32).astype(ml_dtypes.bfloat16)
    xs = xb.reshape(N_SAMPLES, C, H, W_SP)
    return [
        {
            "x": np.ascontiguousarray(xs[i * SPC:(i + 1) * SPC]).reshape(SPC * C, H, W_SP),
            "wd": wd_host,
            "wv": wv_host,
        }
        for i in range(N_CORES)
    ]


def kernel(x: np.ndarray, W: np.ndarray) -> np.ndarray:
    x = np.ascontiguousarray(x, dtype=np.float32)
    W = np.ascontiguousarray(W, dtype=np.float32)
    assert x.shape == (S, B, C, H, W_SP)
    assert W.shape == (C, C, 3, 3)

    nc = _get_program()
    res = run_bass_kernel_spmd(nc, _in_maps(x, W), core_ids=list(range(N_CORES)))
    out = np.concatenate(
        [res.results[i]["out"].reshape(SPC, C, H, W_SP).astype(np.float32)
         for i in range(N_CORES)],
        axis=0,
    )
    return out.reshape(S, B, C, H, W_SP)
